# revision 11
# baseline (speedup 1.0000x reference)
"""Trainium2 Bass kernel for CompositionalCodebookLayer (vector-quantization).

Reference computation (per token t of B*S=8192, per codebook c of 16):
    idx[t,c]  = argmin_n || x[t, c*64:(c+1)*64] - codebook[c, n, :] ||^2
    out[t, c*64:(c+1)*64] = codebook[c, idx[t,c], :]

Device strategy (data-parallel over tokens: 8 cores x 1024 tokens):
  - argmin of distance == argmax of score = dot(x_c, cb_c[n]) - 0.5*||cb_c[n]||^2.
  - The PE computes the scores. Plain fp32 matmul runs at 4 cycles/row on
    TRN2, so the fp32 operands are split into fp16 components (exact
    round-to-nearest splits; two fp16 values carry 22 mantissa bits, i.e.
    nearly a full fp32) and the score comes from 2 full-rate fp16 matmuls
    accumulated in PSUM fp32:
        mmA: [xh;xm]^T  [ch;ch]        (K=128) -> xh.ch + xm.ch
        mmB: [xh;1,1,1]^T [cm;b1,b2,b3] (K=67) -> xh.cm + bias
    Every fp16 product is exact in fp32; dropped cross terms (xm.cm, x.cl,
    xl.*) are ~1e-6 of the score scale, far below typical argmax gaps, so
    the argmax matches the fp32 reference (verified bit-exact on the
    reference seed).
  - Per (128-token tile, codebook): DVE max + max_index give the exact
    first-occurrence argmax of the PSUM scores; an indirect DMA gathers the
    winning code vectors from HBM into the output tile.
"""

import numpy as np

B, S, D = 4, 2048, 1024
C, N, SUB = 16, 2048, 64
NCORES = 8
TOK = B * S              # 8192 tokens
TPC = TOK // NCORES      # 1024 tokens per core
P = 128                  # tokens per tile (partition dim)
NTILES = TPC // P        # 8 tiles per core
NCHUNK = 512             # one PSUM bank of fp32 output per matmul
KA = 2 * SUB             # mmA contraction: xh;xm
KB = SUB + 3             # mmB contraction: xh;ones(3)

_CACHE = {}


def _build_program():
    import concourse.bacc as bacc
    import concourse.mybir as mybir
    import concourse.tile as tile
    from concourse.bass import IndirectOffsetOnAxis

    f32 = mybir.dt.float32
    f16 = mybir.dt.float16

    nc = bacc.Bacc(
        "TRN2",
        target_bir_lowering=False,
        debug=False,
        enable_asserts=False,
        num_devices=NCORES,
    )

    # x operands pre-tiled on host: [tile, K, C*P] so each DMA slice is
    # contiguous 4KB rows per partition.
    xa_d = nc.dram_tensor("xa", [NTILES, KA, C * P], f16, kind="ExternalInput").ap()
    xb_d = nc.dram_tensor("xb", [NTILES, KB, C * P], f16, kind="ExternalInput").ap()
    ca_d = nc.dram_tensor("ca", [KA, C * N], f16, kind="ExternalInput").ap()
    cb_d = nc.dram_tensor("cb", [KB, C * N], f16, kind="ExternalInput").ap()
    g_d = [
        nc.dram_tensor(f"g{c}", [N, SUB], f32, kind="ExternalInput").ap()
        for c in range(C)
    ]
    y_d = nc.dram_tensor("y", [TPC, D], f32, kind="ExternalOutput").ap()

    with tile.TileContext(nc) as tc:
        with (
            tc.tile_pool(name="cbt", bufs=1) as cbt_pool,
            tc.tile_pool(name="xt", bufs=2) as xt_pool,
            tc.tile_pool(name="out", bufs=2) as out_pool,
            tc.tile_pool(name="small", bufs=8) as small_pool,
            tc.tile_pool(name="psum", bufs=2, space="PSUM") as psum_pool,
        ):
            # Codebook operands, SBUF-resident for the whole kernel.
            ca_sb = cbt_pool.tile([KA, C * N], f16)   # [ch; ch]
            cb_sb = cbt_pool.tile([KB, C * N], f16)   # [cm; b1; b2; b3]
            # Per-codebook slice DMAs so codebook 0's matmuls only wait for
            # 1/16th of the load.
            for c in range(C):
                cs = slice(c * N, (c + 1) * N)
                nc.sync.dma_start(ca_sb[:, cs], ca_d[:, cs])
                nc.sync.dma_start(cb_sb[:, cs], cb_d[:, cs])

            for t in range(NTILES):
                ts = slice(t * P, (t + 1) * P)
                xa_sb = xt_pool.tile([KA, C * P], f16, tag="xa")
                xb_sb = xt_pool.tile([KB, C * P], f16, tag="xb")
                for q in range(4):
                    qs = slice(q * 4 * P, (q + 1) * 4 * P)
                    nc.sync.dma_start(xa_sb[:, qs], xa_d[t][:, qs])
                    nc.sync.dma_start(xb_sb[:, qs], xb_d[t][:, qs])
                out_sb = out_pool.tile([P, D], f32)
                for c in range(C):
                    ps = psum_pool.tile([P, N], f32)
                    for k in range(N // NCHUNK):
                        ks = slice(c * N + k * NCHUNK, c * N + (k + 1) * NCHUNK)
                        pk = slice(k * NCHUNK, (k + 1) * NCHUNK)
                        nc.tensor.matmul(
                            ps[:, pk],
                            lhsT=xa_sb[:, c * P : (c + 1) * P],
                            rhs=ca_sb[:, ks],
                            start=True,
                            stop=False,
                        )
                        nc.tensor.matmul(
                            ps[:, pk],
                            lhsT=xb_sb[:, c * P : (c + 1) * P],
                            rhs=cb_sb[:, ks],
                            start=False,
                            stop=True,
                        )
                    mx8 = small_pool.tile([P, 8], f32, tag="mx8")
                    ix8 = small_pool.tile([P, 8], mybir.dt.uint32, tag="ix8")
                    nc.vector.max(out=mx8[:], in_=ps[:])
                    nc.vector.max_index(out=ix8[:], in_max=mx8[:], in_values=ps[:])
                    nc.gpsimd.indirect_dma_start(
                        out=out_sb[:, c * SUB : (c + 1) * SUB],
                        out_offset=None,
                        in_=g_d[c][:],
                        in_offset=IndirectOffsetOnAxis(ap=ix8[:, :1], axis=0),
                    )
                nc.sync.dma_start(y_d[ts, :], out_sb[:])

    nc.compile()
    return nc


def _split16(v):
    """Exact 2-way fp16 split of fp32 array v: v ~= h + m (22 mantissa bits)."""
    h = v.astype(np.float16)
    r = v - h.astype(np.float32)
    m = r.astype(np.float16)
    l = (r - m.astype(np.float32)).astype(np.float16)
    return h, m, l


def _host_prep(x, codebook):
    x = np.ascontiguousarray(x, dtype=np.float32)
    cb = np.ascontiguousarray(codebook, dtype=np.float32)
    xr = x.reshape(TOK, C, SUB)

    ch, cm, _ = _split16(cb)                     # [C, N, SUB] fp16
    bias = -0.5 * np.square(cb).sum(-1)          # [C, N] fp32
    b1, b2, b3 = _split16(bias)

    # ca: [KA, C*N] = [ch; ch];  cb: [KB, C*N] = [cm; b1; b2; b3]
    cht = ch.transpose(2, 0, 1).reshape(SUB, C * N)  # [SUB, C*N]
    ca = np.empty((KA, C * N), dtype=np.float16)
    ca[:SUB] = cht
    ca[SUB:] = cht
    cbm = np.empty((KB, C * N), dtype=np.float16)
    cbm[:SUB] = cm.transpose(2, 0, 1).reshape(SUB, C * N)
    cbm[SUB + 0] = b1.reshape(C * N)
    cbm[SUB + 1] = b2.reshape(C * N)
    cbm[SUB + 2] = b3.reshape(C * N)

    g_ins = {f"g{c}": np.ascontiguousarray(cb[c]) for c in range(C)}

    in_maps = []
    for i in range(NCORES):
        shard = xr[i * TPC : (i + 1) * TPC]      # [TPC, C, SUB]
        xh, xm, _ = _split16(shard)
        # [TPC, C, SUB] -> [NTILES, SUB, C, P]
        def tile_t(v):
            return np.ascontiguousarray(
                v.reshape(NTILES, P, C, SUB).transpose(0, 3, 2, 1)
            )
        xh_t, xm_t = tile_t(xh), tile_t(xm)
        xa = np.empty((NTILES, KA, C * P), dtype=np.float16)
        xa[:, :SUB] = xh_t.reshape(NTILES, SUB, C * P)
        xa[:, SUB:] = xm_t.reshape(NTILES, SUB, C * P)
        xb = np.empty((NTILES, KB, C * P), dtype=np.float16)
        xb[:, :SUB] = xh_t.reshape(NTILES, SUB, C * P)
        xb[:, SUB:] = 1.0
        in_maps.append({"xa": xa, "xb": xb, "ca": ca, "cb": cbm, **g_ins})
    return in_maps


def kernel(x, codebook, _trace=False):
    from concourse.bass_utils import run_bass_kernel_spmd

    if "nc" not in _CACHE:
        _CACHE["nc"] = _build_program()
    nc = _CACHE["nc"]

    in_maps = _host_prep(np.asarray(x), np.asarray(codebook))
    res = run_bass_kernel_spmd(
        nc, in_maps, core_ids=list(range(NCORES)), trace=_trace
    )
    _CACHE["last_result"] = res
    y = np.concatenate([r["y"] for r in res.results], axis=0)  # [TOK, D]
    return y.reshape(B, S, D)


# revision 14
# speedup vs baseline: 1.2080x; 1.2080x over previous
"""Trainium2 Bass kernel for CompositionalCodebookLayer (vector-quantization).

Reference computation (per token t of B*S=8192, per codebook c of 16):
    idx[t,c]  = argmin_n || x[t, c*64:(c+1)*64] - codebook[c, n, :] ||^2
    out[t, c*64:(c+1)*64] = codebook[c, idx[t,c], :]

Device strategy (data-parallel over tokens: 8 cores x 1024 tokens):
  - argmin of distance == argmax of score = dot(x_c, cb_c[n]) - 0.5*||cb_c[n]||^2.
  - The PE computes the scores. Plain fp32 matmul runs at 4 cycles/row on
    TRN2, so the fp32 operands are split into fp16 components (exact
    round-to-nearest splits; two fp16 values carry 22 mantissa bits, i.e.
    nearly a full fp32) and the score comes from 2 full-rate fp16 matmuls
    accumulated in PSUM fp32:
        mmA: [xh;xm]^T  [ch;ch]        (K=128) -> xh.ch + xm.ch
        mmB: [xh;1,1,1]^T [cm;b1,b2,b3] (K=67) -> xh.cm + bias
    Every fp16 product is exact in fp32; dropped cross terms (xm.cm, x.cl,
    xl.*) are ~1e-6 of the score scale, far below typical argmax gaps, so
    the argmax matches the fp32 reference (verified bit-exact on the
    reference seed).
  - Per (128-token tile, codebook): DVE max + max_index give the exact
    first-occurrence argmax of the PSUM scores; an indirect DMA gathers the
    winning code vectors from HBM into the output tile.
"""

import numpy as np

B, S, D = 4, 2048, 1024
C, N, SUB = 16, 2048, 64
NCORES = 8
TOK = B * S              # 8192 tokens
TPC = TOK // NCORES      # 1024 tokens per core
P = 128                  # tokens per tile (partition dim)
NTILES = TPC // P        # 8 tiles per core
NCHUNK = 512             # one PSUM bank of fp32 output per matmul
KA = 2 * SUB             # mmA contraction: xh;xm
KB = SUB + 3             # mmB contraction: xh;ones(3)

_CACHE = {}


def _build_program():
    import concourse.bacc as bacc
    import concourse.mybir as mybir
    import concourse.tile as tile
    from concourse.bass import IndirectOffsetOnAxis

    f32 = mybir.dt.float32
    f16 = mybir.dt.float16

    nc = bacc.Bacc(
        "TRN2",
        target_bir_lowering=False,
        debug=False,
        enable_asserts=False,
        num_devices=NCORES,
    )

    # x operands pre-tiled on host: [tile, K, C*P] so each DMA slice is
    # contiguous 4KB rows per partition.
    xa_d = nc.dram_tensor("xa", [NTILES, KA, C * P], f16, kind="ExternalInput").ap()
    xb_d = nc.dram_tensor("xb", [NTILES, KB, C * P], f16, kind="ExternalInput").ap()
    ca_d = nc.dram_tensor("ca", [KA, C * N], f16, kind="ExternalInput").ap()
    cb_d = nc.dram_tensor("cb", [KB, C * N], f16, kind="ExternalInput").ap()
    g_d = [
        nc.dram_tensor(f"g{c}", [N, SUB], f32, kind="ExternalInput").ap()
        for c in range(C)
    ]
    y_d = nc.dram_tensor("y", [TPC, D], f32, kind="ExternalOutput").ap()

    with tile.TileContext(nc) as tc:
        with (
            tc.tile_pool(name="cbt", bufs=1) as cbt_pool,
            tc.tile_pool(name="xt", bufs=2) as xt_pool,
            tc.tile_pool(name="out", bufs=2) as out_pool,
            tc.tile_pool(name="small", bufs=8) as small_pool,
            tc.tile_pool(name="psum", bufs=2, space="PSUM") as psum_pool,
        ):
            # First tile's x operands land first so matmuls start early.
            xa_sbs, xb_sbs = {}, {}
            xa_sbs[0] = xt_pool.tile([KA, C * P], f16, tag="xa", name="xa_sb0")
            xb_sbs[0] = xt_pool.tile([KB, C * P], f16, tag="xb", name="xb_sb0")
            nc.gpsimd.dma_start(xa_sbs[0][:], xa_d[0])
            nc.gpsimd.dma_start(xb_sbs[0][:], xb_d[0])

            # Codebook operands, SBUF-resident for the whole kernel.
            # Per-codebook slice DMAs (SWDGE) so codebook c's matmuls only
            # wait for the slices loaded so far.
            ca_sb = cbt_pool.tile([KA, C * N], f16)   # [ch; ch]
            cb_sb = cbt_pool.tile([KB, C * N], f16)   # [cm; b1; b2; b3]
            for c in range(C):
                cs = slice(c * N, (c + 1) * N)
                nc.gpsimd.dma_start(ca_sb[:, cs], ca_d[:, cs])
                nc.gpsimd.dma_start(cb_sb[:, cs], cb_d[:, cs])

            # Software-pipelined main loop: DVE max(i) runs while the
            # (i-1)-th max_index is still pending, so the ~2us DVE pipeline
            # drain after MAX8 is hidden under useful work instead of a
            # stall before the dependent FIND_INDEX8.
            pending = None  # (ps, mx8, out_slice, cb_idx, out_dma_args)
            out_sbs = {}
            for t in range(NTILES):
                if t + 1 < NTILES:
                    xa_sbs[t + 1] = xt_pool.tile([KA, C * P], f16, tag="xa", name=f"xa_sb{t+1}")
                    xb_sbs[t + 1] = xt_pool.tile([KB, C * P], f16, tag="xb", name=f"xb_sb{t+1}")
                    nc.gpsimd.dma_start(xa_sbs[t + 1][:], xa_d[t + 1])
                    nc.gpsimd.dma_start(xb_sbs[t + 1][:], xb_d[t + 1])
                xa_sb, xb_sb = xa_sbs.pop(t), xb_sbs.pop(t)
                out_sbs[t] = out_pool.tile([P, D], f32, tag="out_sb", name=f"out_sb{t}")
                for c in range(C):
                    ps = psum_pool.tile([P, N], f32)
                    for k in range(N // NCHUNK):
                        ks = slice(c * N + k * NCHUNK, c * N + (k + 1) * NCHUNK)
                        pk = slice(k * NCHUNK, (k + 1) * NCHUNK)
                        nc.tensor.matmul(
                            ps[:, pk],
                            lhsT=xa_sb[:, c * P : (c + 1) * P],
                            rhs=ca_sb[:, ks],
                            start=True,
                            stop=False,
                        )
                        nc.tensor.matmul(
                            ps[:, pk],
                            lhsT=xb_sb[:, c * P : (c + 1) * P],
                            rhs=cb_sb[:, ks],
                            start=False,
                            stop=True,
                        )
                    mx8 = small_pool.tile([P, 8], f32, tag="mx8")
                    nc.vector.max(out=mx8[:], in_=ps[:])
                    if pending is not None:
                        _drain(nc, tc, small_pool, mybir, IndirectOffsetOnAxis,
                               g_d, y_d, out_sbs, pending)
                    pending = (ps, mx8, t, c)
                # after finishing a tile's 16 codebooks, the final gather for
                # (t, 15) is still pending; the output DMA for tile t is
                # emitted by _drain when (t, 15) retires.
            _drain(nc, tc, small_pool, mybir, IndirectOffsetOnAxis,
                   g_d, y_d, out_sbs, pending, last=True)

    nc.compile()
    return nc


def _drain(nc, tc, small_pool, mybir, IndirectOffsetOnAxis, g_d, y_d,
           out_sbs, pending, last=False):
    """Retire a pending (ps, mx8, t, c): find_index + gather, and flush the
    output tile when its 16th codebook retires."""
    ps, mx8, t, c = pending
    ix8 = small_pool.tile([P, 8], mybir.dt.uint32, tag="ix8")
    nc.vector.max_index(out=ix8[:], in_max=mx8[:], in_values=ps[:])
    nc.gpsimd.indirect_dma_start(
        out=out_sbs[t][:, c * SUB : (c + 1) * SUB],
        out_offset=None,
        in_=g_d[c][:],
        in_offset=IndirectOffsetOnAxis(ap=ix8[:, :1], axis=0),
    )
    if c == C - 1:
        nc.sync.dma_start(y_d[t * P : (t + 1) * P, :], out_sbs.pop(t)[:])


def _split16(v):
    """Exact 2-way fp16 split of fp32 array v: v ~= h + m (22 mantissa bits)."""
    h = v.astype(np.float16)
    r = v - h.astype(np.float32)
    m = r.astype(np.float16)
    l = (r - m.astype(np.float32)).astype(np.float16)
    return h, m, l


def _host_prep(x, codebook):
    x = np.ascontiguousarray(x, dtype=np.float32)
    cb = np.ascontiguousarray(codebook, dtype=np.float32)
    xr = x.reshape(TOK, C, SUB)

    ch, cm, _ = _split16(cb)                     # [C, N, SUB] fp16
    bias = -0.5 * np.square(cb).sum(-1)          # [C, N] fp32
    b1, b2, b3 = _split16(bias)

    # ca: [KA, C*N] = [ch; ch];  cb: [KB, C*N] = [cm; b1; b2; b3]
    cht = ch.transpose(2, 0, 1).reshape(SUB, C * N)  # [SUB, C*N]
    ca = np.empty((KA, C * N), dtype=np.float16)
    ca[:SUB] = cht
    ca[SUB:] = cht
    cbm = np.empty((KB, C * N), dtype=np.float16)
    cbm[:SUB] = cm.transpose(2, 0, 1).reshape(SUB, C * N)
    cbm[SUB + 0] = b1.reshape(C * N)
    cbm[SUB + 1] = b2.reshape(C * N)
    cbm[SUB + 2] = b3.reshape(C * N)

    g_ins = {f"g{c}": np.ascontiguousarray(cb[c]) for c in range(C)}

    in_maps = []
    for i in range(NCORES):
        shard = xr[i * TPC : (i + 1) * TPC]      # [TPC, C, SUB]
        xh, xm, _ = _split16(shard)
        # [TPC, C, SUB] -> [NTILES, SUB, C, P]
        def tile_t(v):
            return np.ascontiguousarray(
                v.reshape(NTILES, P, C, SUB).transpose(0, 3, 2, 1)
            )
        xh_t, xm_t = tile_t(xh), tile_t(xm)
        xa = np.empty((NTILES, KA, C * P), dtype=np.float16)
        xa[:, :SUB] = xh_t.reshape(NTILES, SUB, C * P)
        xa[:, SUB:] = xm_t.reshape(NTILES, SUB, C * P)
        xb = np.empty((NTILES, KB, C * P), dtype=np.float16)
        xb[:, :SUB] = xh_t.reshape(NTILES, SUB, C * P)
        xb[:, SUB:] = 1.0
        in_maps.append({"xa": xa, "xb": xb, "ca": ca, "cb": cbm, **g_ins})
    return in_maps


def kernel(x, codebook, _trace=False):
    from concourse.bass_utils import run_bass_kernel_spmd

    if "nc" not in _CACHE:
        _CACHE["nc"] = _build_program()
    nc = _CACHE["nc"]

    in_maps = _host_prep(np.asarray(x), np.asarray(codebook))
    res = run_bass_kernel_spmd(
        nc, in_maps, core_ids=list(range(NCORES)), trace=_trace
    )
    _CACHE["last_result"] = res
    y = np.concatenate([r["y"] for r in res.results], axis=0)  # [TOK, D]
    return y.reshape(B, S, D)
